# revision 37
# baseline (speedup 1.0000x reference)
"""Single-head causal attention (B=4, S=4096, E=512, DK=DV=64) on 8 trn2 cores.

Sharding: 2 cores per batch element; each core owns 4 q-groups of 512 rows,
chosen so both cores do identical causal work: core role A owns rows
1024g+512..1024g+1024 (g=0..3), role B owns 1024g..1024g+512. The SPMD
program is identical across cores; per-core differences live purely in the
input data (gathered q rows, causal-mask slabs).

Per q-group g the program processes a static n_k = 8g+8 key tiles of 128.
The last 8 key-tiles of every group are multiplied by per-core mask slabs
(A: 4x keep + 4 triangles, B: 4 triangles + 4 zeros).

Layout trick: everything is computed transposed (d-major) so the softmax
reduction is a free-dim reduction and no on-chip transposes of the big
embedding are needed (the host supplies emb^T). P^T = exp(S^T) is already
the right layout for the PV matmul; the column-sum for softmax
normalisation rides along as a 65th row of an ones-augmented V.
"""

import sys

for _p in ("/opt/trn_rl_repo",):
    if _p not in sys.path:
        sys.path.insert(0, _p)

import numpy as np
import ml_dtypes

import concourse.bass as bass
import concourse.bacc as bacc
import concourse.mybir as mybir
from concourse.bass_utils import run_bass_kernel_spmd
from concourse.tile import TileContext

B, S, E, DK, DV = 4, 4096, 512, 64, 64
P = 128
NCORES = 8
NG = 4          # q-groups per core
QG = 512        # q rows per group
NKV = S // QG   # kv groups (8)
F32 = mybir.dt.float32
F32R = mybir.dt.float32r


def r(ap):
    """matmul-feeding tensors are already float32r-typed."""
    return ap


def build_program():
    nc = bacc.Bacc("TRN2", target_bir_lowering=False, debug=False, num_devices=NCORES)

    embT = nc.declare_dram_parameter("embT", [E, S], F32R, isOutput=False)
    qembT = nc.declare_dram_parameter("qembT", [E, NG * QG], F32R, isOutput=False)
    wkv = nc.declare_dram_parameter("wkv", [E, 2 * DK], F32R, isOutput=False)
    wq = nc.declare_dram_parameter("wq", [E, DK], F32R, isOutput=False)
    bkv = nc.declare_dram_parameter("bkv", [2 * DK, 1], F32, isOutput=False)
    bq = nc.declare_dram_parameter("bq", [DK, 1], F32, isOutput=False)
    masks = nc.declare_dram_parameter("masks", [2, P, 4 * QG], mybir.dt.bfloat16, isOutput=False)
    ident = nc.declare_dram_parameter("ident", [P, P], F32R, isOutput=False)
    out = nc.declare_dram_parameter("out", [NG, QG, DV], F32, isOutput=True)

    with TileContext(nc) as tc:
        with (
            tc.tile_pool(name="singles", bufs=1) as singles,
            tc.tile_pool(name="embt", bufs=3) as embt_pool,
            tc.tile_pool(name="qembt", bufs=2) as qembt_pool,
            tc.tile_pool(name="pt", bufs=4) as pt_pool,
            tc.tile_pool(name="fin", bufs=2) as fin_pool,
            tc.tile_pool(name="res", bufs=2) as res_pool,
            tc.tile_pool(name="ps_s", bufs=2, space="PSUM") as ps_s_pool,
            tc.tile_pool(name="ps_o", bufs=2, space="PSUM") as ps_o_pool,
            tc.tile_pool(name="ps_sm", bufs=2, space="PSUM") as ps_sm_pool,
        ):
            # ---- persistent tensors ----
            kvt = singles.tile([P, S], F32R)            # K^T rows 0:64, V^T rows 64:128
            qt = singles.tile([DK, NG * QG], F32R)      # Q^T
            vn = singles.tile([P, S // P, DK + 1], F32R)  # V natural + ones col
            msk = singles.tile([P, 2, 4 * QG], mybir.dt.bfloat16)
            wkv_sb = singles.tile([P, E // P, 2 * DK], F32R)
            wq_sb = singles.tile([P, E // P, DK], F32R)
            bkv_sb = singles.tile([2 * DK, 1], F32)
            bq_sb = singles.tile([DK, 1], F32)
            id_sb = singles.tile([P, P], F32R)

            nc.vector.memset(vn.rearrange("p a b -> p (a b)").bitcast(F32), 1.0)  # ones col for fused colsum


            def kv_dma(j):
                et = embt_pool.tile([P, E // P, QG], F32R, tag="embt")
                for c in range(E // P):
                    nc.sync.dma_start(
                        out=et[:, c, :],
                        in_=embT[:].rearrange("(c p) t -> p c t", p=P)[:, c, j * QG:(j + 1) * QG],
                    )
                return et

            def kv_proj(j, et):
                pkv = ps_sm_pool.tile([P, QG], F32, tag="sm")
                for c in range(E // P):
                    nc.tensor.matmul(
                        pkv, r(wkv_sb[:, c, :]), r(et[:, c, :]),
                        start=(c == 0), stop=(c == E // P - 1),
                    )
                nc.vector.tensor_scalar_add(kvt[:, j * QG:(j + 1) * QG], pkv, bkv_sb)

            def kv_vnat(j):
                # V natural: transpose V^T 128-token blocks
                pv = ps_sm_pool.tile([P, QG], F32R, tag="sm")
                for s in range(QG // P):
                    nc.tensor.transpose(
                        pv[:, s * DK:(s + 1) * DK],
                        kvt[DK:2 * DK, (j * 4 + s) * P:(j * 4 + s + 1) * P],
                        id_sb[DK:2 * DK, DK:2 * DK],
                    )
                nc.vector.tensor_copy(
                    vn[:, j * 4:(j + 1) * 4, 0:DK],
                    pv[:, 0:4 * DK].rearrange("p (s d) -> p s d", d=DK),
                )

            def q_dma(g):
                qe = qembt_pool.tile([P, E // P, QG], F32R, tag="qembt")
                nc.sync.dma_start(
                    out=qe,
                    in_=qembT[:].rearrange("(c p) t -> p c t", p=P)[:, :, g * QG:(g + 1) * QG],
                )
                return qe

            def q_proj(g, qe):
                pq = ps_sm_pool.tile([P, QG], F32, tag="sm")
                for c in range(E // P):
                    nc.tensor.matmul(
                        pq[0:DK, :], r(wq_sb[:, c, :]), r(qe[:, c, :]),
                        start=(c == 0), stop=(c == E // P - 1),
                    )
                nc.vector.tensor_scalar_add(qt[:, g * QG:(g + 1) * QG], pq[0:DK, :], bq_sb)

            def scores(g, kb):
                ps = ps_s_pool.tile([P, 2 * QG], F32, tag="ps")
                qg = qt[:, g * QG:(g + 1) * QG]
                for i in range(2):
                    kt = 2 * kb + i
                    nc.tensor.matmul(
                        ps[:, i * QG:(i + 1) * QG],
                        r(kvt[0:DK, kt * P:(kt + 1) * P]), r(qg),
                        start=True, stop=True,
                    )
                return ps

            def attention(g, fillers=None, first_ps=None):
                fillers = dict(fillers or {})
                n_k = 8 * g + 8
                n_kb = n_k // 2
                po = ps_o_pool.tile([P, QG], F32, tag="po")

                # software pipeline: S^T runs one kb ahead of exp/PV so the
                # scalar engine's exp stream never waits on the PE
                ps_cur = first_ps if first_ps is not None else scores(g, 0)
                for kb in range(n_kb):
                    pt = pt_pool.tile([P, 2 * QG], F32R, tag="pt")
                    nc.scalar.activation(
                        pt, ps_cur, mybir.ActivationFunctionType.Exp, scale=0.125,
                    )
                    if kb + 1 < n_kb:
                        ps_cur = scores(g, kb + 1)
                    # upcoming prologue pieces ride in the PE FIFO ahead of
                    # this kb's PV, placed at the kb where their DMA data has
                    # already landed (an early slot would stall the FIFO)
                    for f in fillers.pop(kb, ()):
                        f()
                    kt0 = 2 * kb
                    if kt0 >= n_k - 8:
                        kk = kt0 - (n_k - 8)   # 0,2,4,6
                        nc.vector.tensor_mul(
                            pt, pt, msk[:, kk // 4, (kk % 4) * QG:(kk % 4 + 2) * QG]
                        )
                    for i in range(2):
                        kt = 2 * kb + i
                        nc.tensor.matmul(
                            po[0:DV + 1, :],
                            r(vn[:, kt, :]), r(pt[:, i * QG:(i + 1) * QG]),
                            start=(kt == 0), stop=(kt == n_k - 1),
                        )
                for kb in sorted(fillers):
                    for f in fillers[kb]:
                        f()
                # hoist the next group's first S^T ahead of this finalize so
                # its exp stream starts without waiting on the PE FIFO tail
                next_ps = scores(g + 1, 0) if g + 1 < NG else None
                # finalize: transpose O^T -> O, divide by colsum, store
                ot = fin_pool.tile([DV + 1, QG], F32, tag="ot")
                nc.vector.tensor_copy(ot, po[0:DV + 1, :])
                pf = ps_sm_pool.tile([P, QG], F32, tag="sm")
                for s in range(QG // P):
                    nc.tensor.transpose(
                        pf[:, s * (DV + 1):(s + 1) * (DV + 1)],
                        ot[:, s * P:(s + 1) * P],
                        id_sb[0:DV + 1, 0:DV + 1].bitcast(F32),
                    )
                rs = fin_pool.tile([P, QG // P, 1], F32, tag="rs")
                nc.vector.reciprocal(
                    rs, pf[:, 0:4 * (DV + 1)].rearrange("p (s d) -> p s d", d=DV + 1)[:, :, DV:DV + 1]
                )
                res = res_pool.tile([P, QG // P, DV], F32, tag="res")
                for s in range(QG // P):
                    nc.vector.tensor_scalar_mul(
                        res[:, s, :], pf[:, s * (DV + 1):s * (DV + 1) + DV], rs[:, s, :]
                    )
                # all input DMAs are issued up front, so the sync queue is
                # idle by now and HWDGE gives the faster store path
                nc.sync.dma_start(
                    out=out[:][g].rearrange("(s p) d -> p s d", p=P), in_=res
                )
                return next_ps

            # emission schedule: all input DMAs issued up front in the order
            # the compute consumes them; group g's attention carries group
            # g+1's prologue compute as fillers
            et = {}
            qe = {}
            et[0] = kv_dma(0)
            nc.sync.dma_start(out=wkv_sb, in_=wkv[:].rearrange("(c p) m -> p c m", p=P))
            nc.sync.dma_start(out=bkv_sb, in_=bkv[:])
            nc.sync.dma_start(out=bq_sb, in_=bq[:])
            # mask slab 0 must land before group 0's first masked PV
            nc.sync.dma_start(out=msk[:, 0, :], in_=masks[:][0])
            qe[0] = q_dma(0)
            nc.sync.dma_start(out=wq_sb, in_=wq[:].rearrange("(c p) m -> p c m", p=P))
            nc.sync.dma_start(out=id_sb, in_=ident[:])
            et[1] = kv_dma(1)
            nc.sync.dma_start(out=msk[:, 1, :], in_=masks[:][1])
            qe[1] = q_dma(1)
            et[2] = kv_dma(2)
            et[3] = kv_dma(3)
            qe[2] = q_dma(2)
            et[4] = kv_dma(4)
            et[5] = kv_dma(5)
            qe[3] = q_dma(3)
            et[6] = kv_dma(6)
            et[7] = kv_dma(7)
            kv_proj(0, et[0])
            q_proj(0, qe[0])

            def KP(j):
                return lambda: kv_proj(j, et[j])

            def KV(j):
                return lambda: kv_vnat(j)

            def QP(g):
                return lambda: q_proj(g, qe[g])

            slot_plan = {
                0: {0: [KV(0), KP(1)], 1: [KV(1)], 99: [QP(1)]},
                1: {1: [KP(2)], 2: [KV(2)], 4: [KP(3)], 5: [KV(3)], 6: [QP(2)]},
                2: {0: [KP(4)], 1: [KV(4)], 2: [KP(5)], 3: [KV(5)], 4: [QP(3)],
                    6: [KP(6)], 7: [KV(6)], 9: [KP(7)], 10: [KV(7)]},
                3: {},
            }
            nxt = None
            for g in range(NG):
                nxt = attention(g, slot_plan[g], first_ps=nxt)

    nc.compile()
    return nc


_PROGRAM = None


def _get_program():
    global _PROGRAM
    if _PROGRAM is None:
        _PROGRAM = build_program()
    return _PROGRAM


def kernel(embedding_matrix, Wq_w, Wq_b, Wk_w, Wk_b, Wv_w, Wv_b):
    emb = np.asarray(embedding_matrix, dtype=np.float32)
    wq = np.asarray(Wq_w, np.float32)
    wk = np.asarray(Wk_w, np.float32)
    wv = np.asarray(Wv_w, np.float32)
    bq = np.asarray(Wq_b, np.float32)
    bk = np.asarray(Wk_b, np.float32)
    bv = np.asarray(Wv_b, np.float32)

    wkv = np.concatenate([wk, wv], axis=1)                      # [E, 128]
    bkv = np.concatenate([bk, bv])[:, None].copy()              # [128, 1]
    ident = np.eye(P, dtype=np.float32)

    # masks[role][slab m, p, col] ; keep iff p + kt'*128 - C <= j, C=512 for A
    mask_by_role = []
    for role in range(2):
        C = 512 if role == 0 else 0
        m = np.zeros((2, P, 4 * QG), np.float32)
        pp = np.arange(P)[:, None]
        jj = np.arange(QG)[None, :]
        for ktp in range(8):
            keep = (pp + ktp * P - C) <= jj
            m[ktp // 4, :, (ktp % 4) * QG:(ktp % 4 + 1) * QG] = keep.astype(np.float32)
        mask_by_role.append(m.astype(ml_dtypes.bfloat16))

    in_maps = []
    for c in range(NCORES):
        b, role = c // 2, c % 2
        embT = np.ascontiguousarray(emb[b].T)                   # [E, S]
        q0s = [1024 * g + (512 if role == 0 else 0) for g in range(NG)]
        qembT = np.ascontiguousarray(
            np.concatenate([emb[b][q0:q0 + QG] for q0 in q0s], axis=0).T
        )                                                       # [E, 2048]
        in_maps.append({
            "embT": embT, "qembT": qembT, "wkv": wkv, "wq": wq,
            "bkv": bkv, "bq": bq[:, None].copy(),
            "masks": mask_by_role[role], "ident": ident,
        })

    nc = _get_program()
    results = run_bass_kernel_spmd(nc, in_maps, list(range(NCORES))).results

    out = np.empty((B, S, DV), np.float32)
    for c in range(NCORES):
        b, role = c // 2, c % 2
        o = results[c]["out"]                                   # [NG, 512, 64]
        for g in range(NG):
            q0 = 1024 * g + (512 if role == 0 else 0)
            out[b, q0:q0 + QG] = o[g]
    return out


if __name__ == "__main__":
    rng = np.random.default_rng(0)
    ins = {
        "embedding_matrix": rng.standard_normal((B, S, E), dtype=np.float32),
        "Wq_w": rng.standard_normal((E, DK), dtype=np.float32) * 0.04,
        "Wq_b": rng.standard_normal((DK,), dtype=np.float32) * 0.04,
        "Wk_w": rng.standard_normal((E, DK), dtype=np.float32) * 0.04,
        "Wk_b": rng.standard_normal((DK,), dtype=np.float32) * 0.04,
        "Wv_w": rng.standard_normal((E, DV), dtype=np.float32) * 0.04,
        "Wv_b": rng.standard_normal((DV,), dtype=np.float32) * 0.04,
    }
    o = kernel(**ins)
    print("kernel ran, out:", o.shape, o.dtype, float(np.abs(o).max()))
